# revision 26
# baseline (speedup 1.0000x reference)
# Causal self-attention (B=2, S=2048, D=1024, H=16) on 8 TRN2 NeuronCores.
#
# Sharding: core = (batch b, head-group hg) with 4 heads per core — data
# parallel on B (cores 0-3 = batch 0, cores 4-7 = batch 1), tensor parallel
# on heads within each batch group. Per core:
#   1. qkv^T projection for its 4 heads:  qkvT[768, 2048] = wqkv_s^T @ x_b^T
#   2. causal attention in scores^T layout (keys on partitions):
#        scoresT[k, q] = K^T.T @ Q^T ; exp on ACT (scale=1/8 fused);
#        diagonal-block masking via an upper-triangular mask multiply;
#        ctxT[d, q]   = [V | 1]^T-style ones-column trick gives softmax
#        denominators in row 64 of the ctx psum.
#   3. normalize ctx^T by the per-(head, q) sums, cast to bf16
#   4. AllGather ctx^T over the 4-core batch group -> full [1024, 2048]
#   5. out-projection for this core's 256 output columns (+bias)
# Host side shards/pre-transposes inputs and concatenates the 8 output
# column-slices; no host arithmetic beyond dtype casts and transposes.

import numpy as np
import ml_dtypes

import concourse.bass as bass
import concourse.mybir as mybir
import concourse.tile as tile
from concourse import bacc
from concourse.bass_utils import run_bass_kernel_spmd
from concourse.masks import make_identity, make_upper_triangular

F32 = mybir.dt.float32
BF16 = mybir.dt.bfloat16

B, S, D, H, HD = 2, 2048, 1024, 16, 64
HG = 4                 # heads per core
DG = HG * HD           # 256 qkv cols per head-group
NCORES = 8
KT = 128               # key tile (partition dim of scoresT)
QC = 512               # query chunk (free dim of scoresT / psum width)
NKT = S // KT          # 16 key tiles
NQC = S // QC          # 4 query chunks
SM_SCALE = 1.0 / 8.0   # 1/sqrt(HD)

# dtype knobs (matmul operand / storage dtypes; psums always fp32)
XW_DT = BF16           # x, w_qkv, and the Q^T/K^T tiles (scores matmul)
V_DT = BF16            # V natural tiles (ctx matmul lhsT)
ATTN_DT = BF16         # exp(scores) tiles (ctx matmul rhs)
CC_DT = BF16           # allgathered ctx^T
WOUT_DT = BF16         # out-projection weights

_NP = {BF16: ml_dtypes.bfloat16, F32: np.float32}

LAST_RESULTS = None    # BassKernelResults of the most recent kernel() call
_NC_CACHE = {}
DEBUG_OUTPUTS = False  # add per-stage debug outputs (dbg_qk/dbg_v/dbg_ctx/...)


def _build_nc():
    nc = bacc.Bacc(
        trn_type="TRN2",
        target_bir_lowering=False,
        debug=False,
        num_devices=NCORES,
    )

    xT = nc.declare_dram_parameter("xT", [D, S], XW_DT, isOutput=False)
    wqkv = nc.declare_dram_parameter("wqkv", [D, 3 * DG], XW_DT, isOutput=False)
    bqkv = nc.declare_dram_parameter("bqkv", [128, 6], F32, isOutput=False)
    wout = nc.declare_dram_parameter("wout", [D, DG], WOUT_DT, isOutput=False)
    bout = nc.declare_dram_parameter("bout", [128, 2], F32, isOutput=False)
    outT = nc.declare_dram_parameter("outT", [DG, S], F32, isOutput=True)
    if DEBUG_OUTPUTS:
        dbg_qk = nc.declare_dram_parameter(
            "dbg_qk", [128, 4 * S], BF16, isOutput=True)
        dbg_v = nc.declare_dram_parameter(
            "dbg_v", [128, HG * NKT * (HD + 1)], BF16, isOutput=True)
        dbg_ctx = nc.declare_dram_parameter(
            "dbg_ctx", [128, 2 * S], BF16, isOutput=True)
        dbg_g = nc.declare_dram_parameter(
            "dbg_g", [128, 8 * S], BF16, isOutput=True)
        dbg_at = nc.declare_dram_parameter(
            "dbg_at", [128, QC], BF16, isOutput=True)
        dbg_raw = nc.declare_dram_parameter(
            "dbg_raw", [64, QC], F32, isOutput=True)
        dbg_sum = nc.declare_dram_parameter(
            "dbg_sum", [1, QC], F32, isOutput=True)
        dbg_recip = nc.declare_dram_parameter(
            "dbg_recip", [1, QC], F32, isOutput=True)

    KC = D // 128  # 8 contraction chunks for the projections

    with tile.TileContext(nc) as tc:
        with tc.tile_pool(name="persist", bufs=1) as ps:
            # ---- constants ----
            identity = ps.tile([128, 128], XW_DT, tag="identity")
            make_identity(nc, identity)
            tri = ps.tile([128, 128], F32, tag="tri")
            make_upper_triangular(nc, tri, val=1.0, diag=True)
            tri_mm = ps.tile([128, 128], ATTN_DT, tag="tri_mm")
            nc.vector.tensor_copy(tri_mm, tri)
            ones1 = ps.tile([1, 64], F32, tag="ones1")
            nc.vector.memset(ones1, 1.0)

            # ---- persistent SBUF tensors ----
            xT_sb = ps.tile([128, KC, S], XW_DT, tag="xT_sb")
            wqkv_sb = ps.tile([128, KC, 3 * DG], XW_DT, tag="wqkv_sb")
            bqkv_sb = ps.tile([128, 6], F32, tag="bqkv_sb")
            qk_sb = ps.tile([128, 4, S], XW_DT, tag="qk_sb")      # Q^T,K^T (4 m-chunks)
            vT_sb = ps.tile([128, 2, S], V_DT, tag="vT_sb")       # V^T (2 m-chunks)
            vnat_sb = ps.tile([128, HG, NKT, HD + 1], V_DT, tag="vnat_sb")
            ctx_sb = ps.tile([128, 2, S], CC_DT, tag="ctx_sb")    # normalized ctx^T
            ctxg_sb = ps.tile([128, D // 128, S], CC_DT, tag="ctxg_sb")
            wout_sb = ps.tile([128, KC, DG], WOUT_DT, tag="wout_sb")
            bout_sb = ps.tile([128, 2], F32, tag="bout_sb")
            outT_sb = ps.tile([128, 2, S], F32, tag="outT_sb")

            # ---- load inputs ----
            xT_r = xT.rearrange("(c p) s -> c p s", p=128)
            wqkv_r = wqkv.rearrange("(c p) m -> c p m", p=128)
            wout_r = wout.rearrange("(c p) m -> c p m", p=128)
            for c in range(KC):
                nc.sync.dma_start(out=xT_sb[:, c, :], in_=xT_r[c])
                nc.sync.dma_start(out=wqkv_sb[:, c, :], in_=wqkv_r[c])
                nc.sync.dma_start(out=wout_sb[:, c, :], in_=wout_r[c])
            nc.sync.dma_start(out=bqkv_sb, in_=bqkv[:])
            nc.sync.dma_start(out=bout_sb, in_=bout[:])

            # ---- stage 1: qkv^T projection ----
            # qkvT[m*128:(m+1)*128, n*512:(n+1)*512], m-chunks: q01 q23 k01 k23 v01 v23
            with tc.tile_pool(name="proj_ps", bufs=4, space="PSUM") as pps, \
                 tc.tile_pool(name="tp_ps", bufs=2, space="PSUM") as tps:
                for m in range(6):
                    for n in range(NQC):
                        pt = pps.tile([128, QC], F32, tag="proj")
                        for c in range(KC):
                            nc.tensor.matmul(
                                pt,
                                lhsT=wqkv_sb[:, c, m * 128:(m + 1) * 128],
                                rhs=xT_sb[:, c, n * QC:(n + 1) * QC],
                                start=(c == 0),
                                stop=(c == KC - 1),
                            )
                        if m < 4:
                            dst = qk_sb[:, m, n * QC:(n + 1) * QC]
                        else:
                            dst = vT_sb[:, m - 4, n * QC:(n + 1) * QC]
                        nc.scalar.activation(
                            dst, pt,
                            mybir.ActivationFunctionType.Identity,
                            bias=bqkv_sb[:, m:m + 1],
                        )

                # ---- stage 2: V natural (+ ones column) via PE transpose ----
                nc.vector.memset(vnat_sb, 1.0)
                for h in range(HG):
                    po = 64 * (h % 2)
                    mv = h // 2
                    for t in range(NKT):
                        tp = tps.tile([128, HD], V_DT, tag="tp")
                        nc.tensor.transpose(
                            tp,
                            vT_sb[po:po + 64, mv, t * KT:(t + 1) * KT],
                            identity[po:po + 64, po:po + 64],
                        )
                        nc.vector.tensor_copy(vnat_sb[:, h, t, 0:HD], tp)

            # ---- stage 3: attention (scores^T layout) ----
            with tc.tile_pool(name="sc_ps", bufs=3, space="PSUM") as scp, \
                 tc.tile_pool(name="ctx_ps", bufs=2, space="PSUM") as cxp, \
                 tc.tile_pool(name="bc_ps", bufs=2, space="PSUM") as bcp, \
                 tc.tile_pool(name="attn_sb", bufs=3) as asb, \
                 tc.tile_pool(name="small_sb", bufs=2) as ssb:
                for j in range(NQC):           # query chunks
                    for h in range(HG):        # heads
                        po = 64 * (h % 2)
                        mh = h // 2
                        cx = cxp.tile([HD + 1, QC], F32, tag="ctx")
                        n_kt = 4 * j + 4      # key tiles 0 .. 4j+3
                        for i in range(n_kt):
                            tshift = KT * i - QC * j
                            t0 = max(tshift, 0)
                            nv = QC - t0
                            sc = scp.tile([128, QC], F32, tag="sc")
                            nc.tensor.matmul(
                                sc[:, 0:nv],
                                lhsT=qk_sb[po:po + 64, 2 + mh, i * KT:(i + 1) * KT],
                                rhs=qk_sb[po:po + 64, mh, j * QC + t0:(j + 1) * QC],
                                start=True, stop=True,
                            )
                            at = asb.tile([128, QC], ATTN_DT, tag="attn")
                            nc.scalar.activation(
                                at[:, 0:nv], sc[:, 0:nv],
                                mybir.ActivationFunctionType.Exp,
                                scale=SM_SCALE,
                            )
                            if tshift >= 0:   # diagonal block: mask k > q
                                nc.vector.tensor_mul(
                                    at[:, 0:128], at[:, 0:128], tri_mm)
                            if DEBUG_OUTPUTS and j == 0 and h == 0 and i == 0:
                                nc.sync.dma_start(out=dbg_at[:], in_=at)
                            nc.tensor.matmul(
                                cx[:, t0:QC],
                                lhsT=vnat_sb[:, h, i, :],
                                rhs=at[:, 0:nv],
                                start=(i == 0),
                                stop=(i == n_kt - 1),
                            )
                        # normalize: recip of sums row, broadcast via K=1
                        # outer product, multiply into ctx_sb (bf16 cast)
                        recip = ssb.tile([1, QC], F32, tag="recip", bufs=4)
                        nc.vector.reciprocal(recip, cx[HD:HD + 1, :])
                        raw = ssb.tile([64, QC], F32, tag="raw", bufs=3)
                        nc.vector.tensor_copy(raw, cx[0:HD, :])
                        bc = bcp.tile([64, QC], F32, tag="bc")
                        nc.tensor.matmul(
                            bc, lhsT=ones1, rhs=recip, start=True, stop=True)
                        nc.vector.tensor_mul(
                            ctx_sb[po:po + 64, mh, j * QC:(j + 1) * QC],
                            raw, bc)
                        if DEBUG_OUTPUTS and j == 0 and h == 0:
                            nc.sync.dma_start(out=dbg_raw[:], in_=raw)
                            nc.sync.dma_start(out=dbg_recip[:], in_=recip)
                            dsum = ssb.tile([1, QC], F32, tag="dsum")
                            nc.vector.tensor_copy(dsum, cx[HD:HD + 1, :])
                            nc.sync.dma_start(out=dbg_sum[:], in_=dsum)

            # ---- stage 4: AllGather ctx^T across the batch group ----
            with tc.tile_pool(name="dram", bufs=1, space="DRAM") as dram:
                cc_in = dram.tile([DG, S], CC_DT, tag="cc_in")
                cc_out = dram.tile([D, S], CC_DT, tag="cc_out")
                cc_in_r = cc_in.rearrange("(c p) s -> c p s", p=128)
                for c in range(2):
                    nc.sync.dma_start(out=cc_in_r[c], in_=ctx_sb[:, c, :])
                nc.gpsimd.collective_compute(
                    "AllGather",
                    mybir.AluOpType.bypass,
                    replica_groups=[[0, 1, 2, 3], [4, 5, 6, 7]],
                    ins=[cc_in[:].opt()],
                    outs=[cc_out[:].opt()],
                )
                cc_out_r = cc_out.rearrange("(c p) s -> c p s", p=128)
                for c in range(D // 128):
                    nc.sync.dma_start(out=ctxg_sb[:, c, :], in_=cc_out_r[c])

            # ---- stage 5: out-projection (this core's 256 columns) ----
            with tc.tile_pool(name="op_ps", bufs=4, space="PSUM") as opp:
                for mo in range(2):
                    for n in range(NQC):
                        pt = opp.tile([128, QC], F32, tag="op")
                        for c in range(KC):
                            nc.tensor.matmul(
                                pt,
                                lhsT=wout_sb[:, c, mo * 128:(mo + 1) * 128],
                                rhs=ctxg_sb[:, c, n * QC:(n + 1) * QC],
                                start=(c == 0),
                                stop=(c == KC - 1),
                            )
                        nc.scalar.activation(
                            outT_sb[:, mo, n * QC:(n + 1) * QC], pt,
                            mybir.ActivationFunctionType.Identity,
                            bias=bout_sb[:, mo:mo + 1],
                        )
                outT_r = outT.rearrange("(c p) s -> c p s", p=128)
                for c in range(2):
                    nc.sync.dma_start(out=outT_r[c], in_=outT_sb[:, c, :])

            if DEBUG_OUTPUTS:
                nc.sync.dma_start(
                    out=dbg_qk[:], in_=qk_sb.rearrange("p c s -> p (c s)"))
                nc.sync.dma_start(
                    out=dbg_v[:], in_=vnat_sb.rearrange("p h t d -> p (h t d)"))
                nc.sync.dma_start(
                    out=dbg_ctx[:], in_=ctx_sb.rearrange("p c s -> p (c s)"))
                nc.sync.dma_start(
                    out=dbg_g[:], in_=ctxg_sb.rearrange("p c s -> p (c s)"))

    nc.compile()
    return nc


def get_nc():
    if "nc" not in _NC_CACHE:
        _NC_CACHE["nc"] = _build_nc()
    return _NC_CACHE["nc"]


def make_in_maps(x, w_qkv, b_qkv, w_out, b_out):
    x = np.asarray(x, np.float32)
    w_qkv = np.asarray(w_qkv, np.float32)
    b_qkv = np.asarray(b_qkv, np.float32)
    w_out = np.asarray(w_out, np.float32)
    b_out = np.asarray(b_out, np.float32)

    xw_np = _NP[XW_DT]
    wout_np = _NP[WOUT_DT]

    xT = [np.ascontiguousarray(x[b].T).astype(xw_np) for b in range(B)]
    in_maps = []
    for core in range(NCORES):
        b, hg = core // HG, core % HG
        sl = slice(hg * DG, (hg + 1) * DG)
        wq = w_qkv[:, sl]
        wk = w_qkv[:, D + hg * DG:D + (hg + 1) * DG]
        wv = w_qkv[:, 2 * D + hg * DG:2 * D + (hg + 1) * DG]
        wqkv_s = np.ascontiguousarray(
            np.concatenate([wq, wk, wv], axis=1)).astype(xw_np)
        bq = np.concatenate(
            [b_qkv[sl], b_qkv[D + hg * DG:D + (hg + 1) * DG],
             b_qkv[2 * D + hg * DG:2 * D + (hg + 1) * DG]])
        in_maps.append({
            "xT": xT[b],
            "wqkv": wqkv_s,
            "bqkv": np.ascontiguousarray(bq.reshape(6, 128).T).astype(np.float32),
            "wout": np.ascontiguousarray(w_out[:, sl]).astype(wout_np),
            "bout": np.ascontiguousarray(
                b_out[sl].reshape(2, 128).T).astype(np.float32),
        })
    return in_maps


def assemble_output(results):
    out = np.empty((B, S, D), np.float32)
    for core in range(NCORES):
        b, hg = core // HG, core % HG
        out[b, :, hg * DG:(hg + 1) * DG] = results[core]["outT"].T
    return out


def kernel(x, w_qkv, b_qkv, w_out, b_out):
    global LAST_RESULTS
    in_maps = make_in_maps(x, w_qkv, b_qkv, w_out, b_out)
    nc = get_nc()
    res = run_bass_kernel_spmd(nc, in_maps, list(range(NCORES)))
    LAST_RESULTS = res
    return assemble_output(res.results)


# revision 31
# speedup vs baseline: 1.2585x; 1.2585x over previous
# Causal self-attention (B=2, S=2048, D=1024, H=16) on 8 TRN2 NeuronCores.
#
# Sharding: core = (batch b, head-group hg) with 4 heads per core — data
# parallel on B (cores 0-3 = batch 0, cores 4-7 = batch 1), tensor parallel
# on heads within each batch group. Per core:
#   1. qkv^T projection for its 4 heads:  qkvT[768, 2048] = wqkv_s^T @ x_b^T
#   2. causal attention in scores^T layout (keys on partitions):
#        scoresT[k, q] = K^T.T @ Q^T ; exp on ACT (scale=1/8 fused, k-tiles
#        exp'd in pairs to amortize the ~352-cycle ACT op overhead);
#        diagonal-block masking via an upper-triangular mask multiply;
#        ctxT[d, q] accumulates with a ones-column appended to V so row 64
#        of the ctx psum is the softmax denominator.
#   3. normalize: broadcast sums via a K=1 outer-product matmul, then a
#      single DVE divide into bf16 ctx_sb
#   4. AllGather ctx^T over the 4-core batch group, split into two token
#      halves so gather #1 overlaps attention of the second half
#   5. out-projection for this core's 256 output columns (+bias), token
#      chunks ordered so the first half starts as soon as gather #1 lands
# Host side shards/pre-transposes inputs and concatenates the 8 output
# column-slices; no host arithmetic beyond dtype casts and transposes.

import numpy as np
import ml_dtypes

import concourse.bass as bass
import concourse.mybir as mybir
import concourse.tile as tile
from concourse import bacc
from concourse.bass_utils import run_bass_kernel_spmd
from concourse.masks import make_identity, make_upper_triangular

F32 = mybir.dt.float32
BF16 = mybir.dt.bfloat16

B, S, D, H, HD = 2, 2048, 1024, 16, 64
HG = 4                 # heads per core
DG = HG * HD           # 256 qkv cols per head-group
NCORES = 8
KT = 128               # key tile (partition dim of scoresT)
QC = 512               # query chunk (free dim of scoresT / psum width)
NKT = S // KT          # 16 key tiles
NQC = S // QC          # 4 query chunks
SM_SCALE = 1.0 / 8.0   # 1/sqrt(HD)

# dtype knobs (matmul operand / storage dtypes; psums always fp32)
XW_DT = BF16           # x, w_qkv, and the Q^T/K^T tiles (scores matmul)
V_DT = BF16            # V natural tiles (ctx matmul lhsT)
ATTN_DT = BF16         # exp(scores) tiles (ctx matmul rhs)
CC_DT = BF16           # allgathered ctx^T
WOUT_DT = BF16         # out-projection weights

_NP = {BF16: ml_dtypes.bfloat16, F32: np.float32}

LAST_RESULTS = None    # BassKernelResults of the most recent kernel() call
_NC_CACHE = {}
DEBUG_OUTPUTS = False  # add per-stage debug outputs (dbg_qk/dbg_v/dbg_ctx/...)


def _build_nc():
    nc = bacc.Bacc(
        trn_type="TRN2",
        target_bir_lowering=False,
        debug=False,
        num_devices=NCORES,
    )

    xT = nc.declare_dram_parameter("xT", [D, S], XW_DT, isOutput=False)
    wqkv = nc.declare_dram_parameter("wqkv", [D, 3 * DG], XW_DT, isOutput=False)
    bqkv = nc.declare_dram_parameter("bqkv", [128, 6], F32, isOutput=False)
    wout = nc.declare_dram_parameter("wout", [D, DG], WOUT_DT, isOutput=False)
    bout = nc.declare_dram_parameter("bout", [128, 2], F32, isOutput=False)
    outT = nc.declare_dram_parameter("outT", [DG, S], F32, isOutput=True)
    if DEBUG_OUTPUTS:
        dbg_qk = nc.declare_dram_parameter(
            "dbg_qk", [128, 4 * S], BF16, isOutput=True)
        dbg_v = nc.declare_dram_parameter(
            "dbg_v", [128, HG * NKT * (HD + 1)], BF16, isOutput=True)
        dbg_ctx = nc.declare_dram_parameter(
            "dbg_ctx", [128, 2 * S], BF16, isOutput=True)
        dbg_g = nc.declare_dram_parameter(
            "dbg_g", [128, 8 * S], BF16, isOutput=True)

    KC = D // 128  # 8 contraction chunks for the projections

    with tile.TileContext(nc) as tc:
        with tc.tile_pool(name="persist", bufs=1) as ps:
            # ---- constants ----
            identity = ps.tile([128, 128], XW_DT, tag="identity")
            make_identity(nc, identity)
            tri = ps.tile([128, 128], F32, tag="tri")
            make_upper_triangular(nc, tri, val=1.0, diag=True)
            tri_mm = ps.tile([128, 128], ATTN_DT, tag="tri_mm")
            nc.vector.tensor_copy(tri_mm, tri)
            ones1 = ps.tile([1, 64], F32, tag="ones1")
            nc.vector.memset(ones1, 1.0)

            # ---- persistent SBUF tensors ----
            xT_sb = ps.tile([128, KC, S], XW_DT, tag="xT_sb")
            wqkv_sb = ps.tile([128, KC, 3 * DG], XW_DT, tag="wqkv_sb")
            bqkv_sb = ps.tile([128, 6], F32, tag="bqkv_sb")
            qk_sb = ps.tile([128, 4, S], XW_DT, tag="qk_sb")      # Q^T,K^T
            vT_sb = ps.tile([128, 2, S], V_DT, tag="vT_sb")       # V^T
            vnat_sb = ps.tile([128, HG, NKT, HD + 1], V_DT, tag="vnat_sb")
            ctx_sb = ps.tile([128, 2, S], CC_DT, tag="ctx_sb")    # normalized
            ctxg_sb = ps.tile([128, D // 128, S], CC_DT, tag="ctxg_sb")
            wout_sb = ps.tile([128, KC, DG], WOUT_DT, tag="wout_sb")
            bout_sb = ps.tile([128, 2], F32, tag="bout_sb")
            outT_sb = ps.tile([128, 2, S], F32, tag="outT_sb")

            # ---- load inputs ----
            xT_r = xT.rearrange("(c p) s -> c p s", p=128)
            wqkv_r = wqkv.rearrange("(c p) m -> c p m", p=128)
            wout_r = wout.rearrange("(c p) m -> c p m", p=128)
            for c in range(KC):
                nc.sync.dma_start(out=xT_sb[:, c, :], in_=xT_r[c])
                nc.sync.dma_start(out=wqkv_sb[:, c, :], in_=wqkv_r[c])
                nc.sync.dma_start(out=wout_sb[:, c, :], in_=wout_r[c])
            nc.sync.dma_start(out=bqkv_sb, in_=bqkv[:])
            nc.sync.dma_start(out=bout_sb, in_=bout[:])

            # ---- stage 1: qkv^T projection ----
            # m-chunk order: q01 q23 k01 k23 v01 v23
            with tc.tile_pool(name="proj_ps", bufs=4, space="PSUM") as pps, \
                 tc.tile_pool(name="tp_ps", bufs=2, space="PSUM") as tps:
                for m in range(6):
                    for n in range(NQC):
                        pt = pps.tile([128, QC], F32, tag="proj")
                        for c in range(KC):
                            nc.tensor.matmul(
                                pt,
                                lhsT=wqkv_sb[:, c, m * 128:(m + 1) * 128],
                                rhs=xT_sb[:, c, n * QC:(n + 1) * QC],
                                start=(c == 0),
                                stop=(c == KC - 1),
                            )
                        if m < 4:
                            dst = qk_sb[:, m, n * QC:(n + 1) * QC]
                        else:
                            dst = vT_sb[:, m - 4, n * QC:(n + 1) * QC]
                        nc.vector.tensor_scalar_add(
                            dst, pt, bqkv_sb[:, m:m + 1])

                # ---- stage 2: V natural (+ ones column) via PE transpose ----
                nc.vector.memset(vnat_sb, 1.0)
                for h in range(HG):
                    po = 64 * (h % 2)
                    mv = h // 2
                    for t in range(NKT):
                        tp = tps.tile([128, HD], V_DT, tag="tp")
                        nc.tensor.transpose(
                            tp,
                            vT_sb[po:po + 64, mv, t * KT:(t + 1) * KT],
                            identity[po:po + 64, po:po + 64],
                        )
                        nc.vector.tensor_copy(vnat_sb[:, h, t, 0:HD], tp)

            # ---- stages 3-5 interleaved: attention, gather, out-proj ----
            with tc.tile_pool(name="dram", bufs=1, space="DRAM") as dram:

                cc_in = [dram.tile([DG, S // 2], CC_DT, tag=f"cc_in{half}",
                                   name=f"cc_in{half}") for half in range(2)]
                cc_out = [dram.tile([D, S // 2], CC_DT, tag=f"cc_out{half}",
                                    name=f"cc_out{half}") for half in range(2)]

                def attention_chunk(j):
                    for h in range(HG):
                        po = 64 * (h % 2)
                        mh = h // 2
                        cx = cxp.tile([HD + 1, QC], F32, tag="ctx")
                        n_kt = 4 * j + 4      # key tiles 0 .. 4j+3
                        for i0 in range(0, n_kt, 2):
                            pair = (i0, i0 + 1)
                            sc = scp.tile([128, 2, QC], F32, tag="sc")
                            at = asb.tile([128, 2, QC], ATTN_DT, tag="attn")
                            for kk, i in enumerate(pair):
                                nc.tensor.matmul(
                                    sc[:, kk, :],
                                    lhsT=qk_sb[po:po + 64, 2 + mh,
                                               i * KT:(i + 1) * KT],
                                    rhs=qk_sb[po:po + 64, mh,
                                              j * QC:(j + 1) * QC],
                                    start=True, stop=True,
                                )
                            # one exp over the pair; stale cols below the
                            # diagonal are never read by the ctx matmul
                            nc.scalar.activation(
                                at[:, :, :], sc[:, :, :],
                                mybir.ActivationFunctionType.Exp,
                                scale=SM_SCALE,
                            )
                            for kk, i in enumerate(pair):
                                tshift = KT * i - QC * j
                                t0 = max(tshift, 0)
                                if tshift >= 0:   # diagonal: mask k > q
                                    nc.vector.tensor_mul(
                                        at[:, kk, t0:t0 + 128],
                                        at[:, kk, t0:t0 + 128], tri_mm)
                                nc.tensor.matmul(
                                    cx[:, t0:QC],
                                    lhsT=vnat_sb[:, h, i, :],
                                    rhs=at[:, kk, t0:QC],
                                    start=(i == 0),
                                    stop=(i == n_kt - 1),
                                )
                        # normalize: recip of sums row, broadcast via K=1
                        # outer product, multiply into ctx_sb (bf16 cast)
                        recip = ssb.tile([1, QC], F32, tag="recip", bufs=4)
                        nc.vector.reciprocal(recip, cx[HD:HD + 1, :])
                        raw = ssb.tile([64, QC], F32, tag="raw", bufs=3)
                        nc.vector.tensor_copy(raw, cx[0:HD, :])
                        bc = bcp.tile([64, QC], F32, tag="bc")
                        nc.tensor.matmul(
                            bc, lhsT=ones1, rhs=recip, start=True, stop=True)
                        nc.vector.tensor_mul(
                            ctx_sb[po:po + 64, mh, j * QC:(j + 1) * QC],
                            raw, bc)

                def gather_half(half):
                    lo = half * (S // 2)
                    cc_in_r = cc_in[half].rearrange("(c p) s -> c p s", p=128)
                    for c in range(2):
                        nc.sync.dma_start(
                            out=cc_in_r[c],
                            in_=ctx_sb[:, c, lo:lo + S // 2])
                    nc.gpsimd.collective_compute(
                        "AllGather",
                        mybir.AluOpType.bypass,
                        replica_groups=[[0, 1, 2, 3], [4, 5, 6, 7]],
                        ins=[cc_in[half][:].opt()],
                        outs=[cc_out[half][:].opt()],
                    )
                    cc_out_r = cc_out[half].rearrange(
                        "(c p) s -> c p s", p=128)
                    for c in range(D // 128):
                        nc.sync.dma_start(
                            out=ctxg_sb[:, c, lo:lo + S // 2],
                            in_=cc_out_r[c])

                def out_proj_chunk(n, opp):
                    for mo in range(2):
                        pt = opp.tile([128, QC], F32, tag="op")
                        for c in range(KC):
                            nc.tensor.matmul(
                                pt,
                                lhsT=wout_sb[:, c, mo * 128:(mo + 1) * 128],
                                rhs=ctxg_sb[:, c, n * QC:(n + 1) * QC],
                                start=(c == 0),
                                stop=(c == KC - 1),
                            )
                        nc.vector.tensor_scalar_add(
                            outT_sb[:, mo, n * QC:(n + 1) * QC], pt,
                            bout_sb[:, mo:mo + 1])

                with tc.tile_pool(name="sc_ps", bufs=2, space="PSUM") as scp, \
                     tc.tile_pool(name="ctx_ps", bufs=2, space="PSUM") as cxp, \
                     tc.tile_pool(name="bc_ps", bufs=2, space="PSUM") as bcp, \
                     tc.tile_pool(name="attn_sb", bufs=3) as asb, \
                     tc.tile_pool(name="small_sb", bufs=2) as ssb:
                    attention_chunk(0)
                    attention_chunk(1)
                    gather_half(0)      # overlaps attention of chunks 2,3
                    attention_chunk(2)
                    attention_chunk(3)
                    gather_half(1)
                with tc.tile_pool(name="op_ps", bufs=2, space="PSUM") as opp:
                    out_proj_chunk(0, opp)  # overlaps gather of half 1
                    out_proj_chunk(1, opp)
                    out_proj_chunk(2, opp)
                    out_proj_chunk(3, opp)
                    outT_r = outT.rearrange("(c p) s -> c p s", p=128)
                    for c in range(2):
                        nc.sync.dma_start(out=outT_r[c], in_=outT_sb[:, c, :])

            if DEBUG_OUTPUTS:
                nc.sync.dma_start(
                    out=dbg_qk[:], in_=qk_sb.rearrange("p c s -> p (c s)"))
                nc.sync.dma_start(
                    out=dbg_v[:], in_=vnat_sb.rearrange("p h t d -> p (h t d)"))
                nc.sync.dma_start(
                    out=dbg_ctx[:], in_=ctx_sb.rearrange("p c s -> p (c s)"))
                nc.sync.dma_start(
                    out=dbg_g[:], in_=ctxg_sb.rearrange("p c s -> p (c s)"))

    nc.compile()
    return nc


def get_nc():
    if "nc" not in _NC_CACHE:
        _NC_CACHE["nc"] = _build_nc()
    return _NC_CACHE["nc"]


def make_in_maps(x, w_qkv, b_qkv, w_out, b_out):
    x = np.asarray(x, np.float32)
    w_qkv = np.asarray(w_qkv, np.float32)
    b_qkv = np.asarray(b_qkv, np.float32)
    w_out = np.asarray(w_out, np.float32)
    b_out = np.asarray(b_out, np.float32)

    xw_np = _NP[XW_DT]
    wout_np = _NP[WOUT_DT]

    xT = [np.ascontiguousarray(x[b].T).astype(xw_np) for b in range(B)]
    in_maps = []
    for core in range(NCORES):
        b, hg = core // HG, core % HG
        sl = slice(hg * DG, (hg + 1) * DG)
        wq = w_qkv[:, sl]
        wk = w_qkv[:, D + hg * DG:D + (hg + 1) * DG]
        wv = w_qkv[:, 2 * D + hg * DG:2 * D + (hg + 1) * DG]
        wqkv_s = np.ascontiguousarray(
            np.concatenate([wq, wk, wv], axis=1)).astype(xw_np)
        bq = np.concatenate(
            [b_qkv[sl], b_qkv[D + hg * DG:D + (hg + 1) * DG],
             b_qkv[2 * D + hg * DG:2 * D + (hg + 1) * DG]])
        in_maps.append({
            "xT": xT[b],
            "wqkv": wqkv_s,
            "bqkv": np.ascontiguousarray(bq.reshape(6, 128).T).astype(np.float32),
            "wout": np.ascontiguousarray(w_out[:, sl]).astype(wout_np),
            "bout": np.ascontiguousarray(
                b_out[sl].reshape(2, 128).T).astype(np.float32),
        })
    return in_maps


def assemble_output(results):
    out = np.empty((B, S, D), np.float32)
    for core in range(NCORES):
        b, hg = core // HG, core % HG
        out[b, :, hg * DG:(hg + 1) * DG] = results[core]["outT"].T
    return out


def kernel(x, w_qkv, b_qkv, w_out, b_out):
    global LAST_RESULTS
    in_maps = make_in_maps(x, w_qkv, b_qkv, w_out, b_out)
    nc = get_nc()
    res = run_bass_kernel_spmd(nc, in_maps, list(range(NCORES)))
    LAST_RESULTS = res
    return assemble_output(res.results)
